# revision 4
# baseline (speedup 1.0000x reference)
"""Trainium2 Bass kernel v2 for nn_DiffusionNCA_fft2 (B=32, S=64, C=32, HID=256).

Self-contained: FULL inputs, batch sharded over 8 NeuronCores (4/core).

v2 vs baseline: the two 64B-packet DRAM-bounce transposes (post-F1 and
mid-iFFT) are replaced with XBAR DMA-transposes reading overlapping
strided rows (junk in partitions 64-127 is never consumed); F2/iFFT-B
become split-weight K=64 accumulating matmul pairs; psum evacs on those
paths are contiguous; emission is wavefront-pipelined across the 4
batches so the PE stays HAM-warm and DMA latency hides behind compute.
"""

import os
from contextlib import ExitStack

import numpy as np
import ml_dtypes

import concourse.bass as bass
import concourse.mybir as mybir
import concourse.tile as tile
from concourse import bacc

S = 64
C = 32
C2 = 64
C6 = 192
HID = 256
B = 32
NCORES = 8
BPC = B // NCORES
SP = 66
NPAD = SP * SP
NPIX = S * S
NFLAT = 2 * S * C * S        # 262144
LN_N = float(HID * NPIX)
EPS = 1e-5
FIRE = 0.5

f32 = mybir.dt.float32
bf16 = mybir.dt.bfloat16
AF = mybir.ActivationFunctionType
ALU = mybir.AluOpType

_BF = ml_dtypes.bfloat16
GP_DMA = bool(int(os.environ.get('GP_DMA', '1')))
GP_COMP = bool(int(os.environ.get('GP_COMP', '1')))
GP_ACC = bool(int(os.environ.get('GP_ACC', '0')))


def _dft_mats():
    t = np.arange(S)
    ang = -2.0 * np.pi * np.outer(t, t) / S
    return np.cos(ang).astype(np.float32), np.sin(ang).astype(np.float32)


def host_constants(inp):
    Fr, Fi = _dft_mats()
    cst = {}

    ff1 = np.zeros((S, 2 * S), np.float32)
    ff1[:, :S], ff1[:, S:] = Fr.T, Fi.T
    cst["ff1"] = ff1.astype(_BF)

    w2 = np.zeros((2 * S, 2 * S), np.float32)
    w2[:S, :S], w2[S:, :S] = Fr.T, -Fi.T
    w2[:S, S:], w2[S:, S:] = Fi.T, Fr.T
    cst["w2lo"] = w2[:S, :].astype(_BF)
    cst["w2hi"] = w2[S:, :].astype(_BF)

    Gr, Gi = Fr / S, -Fi / S
    wa = np.zeros((2 * S, 2 * S), np.float32)
    wa[:S, :S], wa[S:, :S] = Gr.T, -Gi.T
    wa[:S, S:], wa[S:, S:] = Gi.T, Gr.T
    cst["wa"] = wa.astype(_BF)
    cst["walo"] = wa[:S, :].astype(_BF)
    cst["wahi"] = wa[S:, :].astype(_BF)

    a = np.linspace(1.0, 0.0, S, dtype=np.float32)
    alive = (a[:, None] + a[None, :]) * 0.5
    cst["alive"] = np.pad(alive, 1, mode="reflect").reshape(-1).astype(_BF)

    p0w, p1w = np.asarray(inp["p0_w"]), np.asarray(inp["p1_w"])
    wpair = np.zeros((2 * C2, 3 * 2 * C2), np.float32)
    wsing = np.zeros((C2, 3 * 2 * C2), np.float32)
    for di in range(3):
        mo = di * 2 * C2
        wpair[:C2, mo:mo + C2] = p0w[:, :, di, 0].T
        wpair[C2:, mo:mo + C2] = p0w[:, :, di, 1].T
        wpair[:C2, mo + C2:mo + 2 * C2] = p1w[:, :, di, 0].T
        wpair[C2:, mo + C2:mo + 2 * C2] = p1w[:, :, di, 1].T
        wsing[:, mo:mo + C2] = p0w[:, :, di, 2].T
        wsing[:, mo + C2:mo + 2 * C2] = p1w[:, :, di, 2].T
    cst["wpair"] = wpair.astype(_BF)
    cst["wsing"] = wsing.astype(_BF)

    fc0w = np.asarray(inp["fc0_w"])
    cst["fc0a"] = fc0w[:C2].astype(_BF)
    cst["fc0bb"] = fc0w[C2:].astype(_BF)
    fc0b = (np.asarray(inp["fc0_b"])
            + np.asarray(inp["p0_b"]) @ fc0w[C2:2 * C2]
            + np.asarray(inp["p1_b"]) @ fc0w[2 * C2:])
    cst["fc0b2"] = fc0b.reshape(2, 128).T.astype(np.float32).copy()

    fc1w = np.asarray(inp["fc1_w"]).astype(np.float32)
    fc1t = np.zeros((128, 128), np.float32)
    fc1t[:, :64], fc1t[:, 64:] = fc1w[:128], fc1w[128:]
    cst["fc1"] = fc1t.astype(_BF)

    lnw = np.asarray(inp["ln_w"]).astype(np.float32)
    lnb = np.asarray(inp["ln_b"]).astype(np.float32)
    lnw_dev = np.transpose(lnw, (2, 1, 0)).reshape(HID, NPIX)
    lnb_dev = np.transpose(lnb, (2, 1, 0)).reshape(HID, NPIX)
    cst["lnw"] = np.concatenate([lnw_dev[:128], lnw_dev[128:]], axis=1).astype(_BF)
    lw1 = fc1w[:128].T @ lnw_dev[:128] + fc1w[128:].T @ lnw_dev[128:]
    lb1 = fc1w[:128].T @ lnb_dev[:128] + fc1w[128:].T @ lnb_dev[128:]
    cst["lw1t"] = np.concatenate([lw1, lw1], axis=0).astype(_BF)
    cst["lbt"] = np.concatenate([lb1, lb1], axis=0).astype(_BF)
    return cst


def build_nc(steps=1):
    nc = bacc.Bacc("TRN2", target_bir_lowering=False, debug=False)

    xs = nc.dram_tensor("xs", [BPC, S, C * S], bf16, kind="ExternalInput")
    ins = {}
    cshape = dict(ff1=([S, 2 * S], bf16),
                  w2lo=([S, 2 * S], bf16), w2hi=([S, 2 * S], bf16),
                  wa=([2 * S, 2 * S], bf16),
                  walo=([S, 2 * S], bf16), wahi=([S, 2 * S], bf16),
                  alive=([NPAD], bf16),
                  wpair=([2 * C2, 3 * 2 * C2], bf16), wsing=([C2, 3 * 2 * C2], bf16),
                  fc0a=([C2, HID], bf16), fc0bb=([2 * C2, HID], bf16),
                  fc0b2=([128, 2], f32), fc1=([128, 128], bf16),
                  lnw=([128, 2 * NPIX], bf16), lw1t=([128, NPIX], bf16),
                  lbt=([128, NPIX], bf16))
    for name, (shp, dt) in cshape.items():
        ins[name] = nc.dram_tensor(name, shp, dt, kind="ExternalInput")
    maskd = nc.dram_tensor("maskd", [BPC // 2, 128, NPIX], bf16, kind="ExternalInput")

    D1 = nc.dram_tensor("D1", [BPC, NFLAT + 64], bf16)
    D2 = nc.dram_tensor("D2", [BPC, 2 * S, C * S], bf16)
    D3 = nc.dram_tensor("D3", [BPC // 2, 2, 2 * S, C * S], bf16)
    D4 = nc.dram_tensor("D4", [BPC, NFLAT + 64], bf16)
    DS = nc.dram_tensor("DS", [BPC, 2], f32)
    OUT = nc.dram_tensor("OUT", [BPC, 2 * S, S * C], bf16, kind="ExternalOutput")

    with tile.TileContext(nc) as tc, ExitStack() as ctx:
        cpool = ctx.enter_context(tc.tile_pool(name="consts", bufs=1))
        xpool = ctx.enter_context(tc.tile_pool(name="x", bufs=2))
        tdpool = ctx.enter_context(tc.tile_pool(name="td", bufs=2))
        tgpool = ctx.enter_context(tc.tile_pool(name="tg", bufs=4))
        s2pool = ctx.enter_context(tc.tile_pool(name="s2p", bufs=2))
        dxpool = ctx.enter_context(tc.tile_pool(name="dx", bufs=2))
        ypool = ctx.enter_context(tc.tile_pool(name="yconv", bufs=2))
        hpool = ctx.enter_context(tc.tile_pool(name="h", bufs=4))
        spool = ctx.enter_context(tc.tile_pool(name="small", bufs=8))
        sqpool = ctx.enter_context(tc.tile_pool(name="sq", bufs=1))
        zpool = ctx.enter_context(tc.tile_pool(name="z", bufs=1))
        stpool = ctx.enter_context(tc.tile_pool(name="stp", bufs=2))
        mpool = ctx.enter_context(tc.tile_pool(name="maskp", bufs=1))
        dmpool = ctx.enter_context(tc.tile_pool(name="dm", bufs=2))
        dgpool = ctx.enter_context(tc.tile_pool(name="dg", bufs=2))
        sapool = ctx.enter_context(tc.tile_pool(name="sa", bufs=2))
        gbpool = ctx.enter_context(tc.tile_pool(name="gb", bufs=4))
        sbpool = ctx.enter_context(tc.tile_pool(name="sb", bufs=1))
        pfft = ctx.enter_context(tc.tile_pool(name="pfft", bufs=2, space="PSUM"))
        pmm = ctx.enter_context(tc.tile_pool(name="pmm", bufs=2, space="PSUM"))

        # constants: light (front-end) ones on sync/scalar first, heavy LN/fc1
        # tail constants on the gpsimd SWDGE ring so they never block the
        # front-end rings.
        ct = {}
        gpd = nc.gpsimd if GP_DMA else nc.scalar
        gpd2 = nc.gpsimd if GP_DMA else nc.sync
        ring = dict(ff1=nc.sync, w2lo=nc.sync, w2hi=nc.sync,
                    wpair=nc.scalar, wsing=nc.scalar, fc0a=nc.scalar,
                    fc0bb=nc.scalar, fc0b2=nc.scalar,
                    wa=gpd, walo=gpd, wahi=gpd,
                    fc1=gpd, lnw=gpd, lw1t=gpd,
                    lbt=gpd)
        order = ["ff1", "w2lo", "w2hi", "wpair", "wsing", "fc0a", "fc0bb",
                 "fc0b2", "wa", "walo", "wahi", "fc1", "lnw", "lw1t", "lbt"]
        for name in order:
            shp, dt = cshape[name]
            t = cpool.tile(shp, dt, tag="c_" + name)
            ring[name].dma_start(t[:], ins[name][:])
            ct[name] = t

        ones = cpool.tile([128, 128], f32, tag="c_ones")
        nc.gpsimd.memset(ones[:], 1.0)

        st = [dict() for _ in range(BPC)]
        stats = {}
        dgath = {}

        def s_load(b):
            X = xpool.tile([S, C * S], bf16, tag="X", name=f"X_{b}")
            nc.sync.dma_start(X[:], xs[b])
            st[b]["X"] = X

        def s_f1(b):
            X = st[b]["X"]
            t1d = tdpool.tile([2 * S, C * S], bf16, tag="t1d", name=f"t1d_{b}")
            for half in range(2):
                ps = pfft.tile([2 * S, 1024], f32, tag="pfft")
                for q in range(2):
                    sl = bass.ds(half * 1024 + q * 512, 512)
                    nc.tensor.matmul(ps[:, bass.ts(q, 512)], ct["ff1"][:], X[:, sl])
                eng = nc.vector.tensor_copy if half == 0 else nc.scalar.copy
                eng(t1d[:, bass.ts(half, 1024)], ps[:])
            nc.sync.dma_start(
                D1[b][0:NFLAT].rearrange("(p f) -> p f", p=2 * S, f=C * S), t1d[:])
            st[b]["t1d"] = t1d

        def s_ta(b):
            for ri in range(2):
                t1g = tgpool.tile([128, S * C], bf16, tag="t1g", name=f"t1g_{b}_{ri}")
                in_ap = bass.AP(D1, b * (NFLAT + 64) + ri * (NFLAT // 2),
                                [[64, 2048], [1, 128]])
                nc.sync.dma_start(t1g[:], in_ap, transpose=True)
                st[b][f"t1g{ri}"] = t1g

        def s_f2(b):
            t1g0, t1g1 = st[b]["t1g0"], st[b]["t1g1"]
            s2 = s2pool.tile([2 * S, C * S], bf16, tag="s2", name=f"s2_{b}")
            for half in range(2):
                ps = pfft.tile([2 * S, 1024], f32, tag="pfft")
                for q in range(2):
                    sl = bass.ds(half * 1024 + q * 512, 512)
                    nc.tensor.matmul(ps[:, bass.ts(q, 512)], ct["w2lo"][:],
                                     t1g0[0:S, sl], start=True, stop=False)
                    nc.tensor.matmul(ps[:, bass.ts(q, 512)], ct["w2hi"][:],
                                     t1g1[0:S, sl], start=False, stop=True)
                eng = nc.scalar.copy if half == 0 else nc.vector.tensor_copy
                eng(s2[:].rearrange("p (c v) -> p v c", c=C, v=S)[:, bass.ts(half, 32), :],
                    ps[:].rearrange("p (v c) -> p v c", v=32, c=C))
            nc.scalar.dma_start(D2[b][:], s2[:])
            st[b]["s2"] = s2

        def s_dx(b):
            dx2 = dxpool.tile([2 * C2, NPAD], bf16, tag="dx2", name=f"dx2_{b}")
            dxv = dx2[:, 0:NPAD].rearrange("q (a b) -> q a b", a=SP, b=SP)
            d2v = D2[b].rearrange("(ri u) (c v) -> ri c u v", ri=2, u=S, c=C, v=S)
            nc.sync.dma_start(dxv[0:32, 1:S + 1, 1:S + 1], d2v[0])
            nc.scalar.dma_start(dxv[32:64, 1:S + 1, 1:S + 1], d2v[1])
            nc.sync.dma_start(dx2[C2 - 1:C2, 0:NPAD], ins["alive"][None, :])
            q = slice(0, C2 - 1)
            nc.vector.tensor_copy(dxv[q, 1:S + 1, 0:1], dxv[q, 1:S + 1, 2:3])
            nc.vector.tensor_copy(dxv[q, 1:S + 1, SP - 1:SP],
                                  dxv[q, 1:S + 1, SP - 3:SP - 2])
            nc.vector.tensor_copy(dxv[q, 0:1, :], dxv[q, 2:3, :])
            nc.vector.tensor_copy(dxv[q, SP - 1:SP, :], dxv[q, SP - 3:SP - 2, :])
            nc.sync.dma_start(dxv[64:96, 1:S + 1, 0:S], d2v[0])
            nc.scalar.dma_start(dxv[96:128, 1:S + 1, 0:S], d2v[1])
            nc.scalar.dma_start(dx2[2 * C2 - 1:2 * C2, 0:NPAD - 1],
                                ins["alive"][None, 1:NPAD])
            qb = slice(C2, 2 * C2 - 1)
            nc.vector.tensor_copy(dxv[qb, 0:1, 0:S], dxv[qb, 2:3, 0:S])
            nc.vector.tensor_copy(dxv[qb, SP - 1:SP, 0:S],
                                  dxv[qb, SP - 3:SP - 2, 0:S])
            st[b]["dx2"] = dx2

        def s_conv(b):
            dx2 = st[b]["dx2"]
            dxv = dx2[:, 0:NPAD].rearrange("q (a b) -> q a b", a=SP, b=SP)
            s1cols = spool.tile([128, 8], f32, tag="s1cols")
            h_tiles = {}
            for m in range(2):
                h_tiles[m] = hpool.tile([128, NPIX], bf16, tag="h", name=f"h_{b}_{m}")
            st[b]["h"] = h_tiles
            for T in range(4):
                r0 = T * 16
                psy = pmm.tile([2 * C2, 1024], f32, tag="pmm")
                for q in range(2):
                    rq = r0 + q * 8
                    for di in range(3):
                        nc.tensor.matmul(
                            psy[:, bass.ts(q, 512)],
                            ct["wpair"][:, bass.ts(di, 2 * C2)],
                            dxv[:, rq + di:rq + di + 8, 0:S],
                            start=(di == 0), stop=False)
                    for di in range(3):
                        nc.tensor.matmul(
                            psy[:, bass.ts(q, 512)],
                            ct["wsing"][:, bass.ts(di, 2 * C2)],
                            dxv[0:C2, rq + di:rq + di + 8, 2:SP],
                            start=False, stop=(di == 2))
                yc = ypool.tile([2 * C2, 1024], bf16, tag="yc")
                eng = nc.vector.tensor_copy if T % 2 == 0 else nc.scalar.copy
                eng(yc[:], psy[:])
                for m in range(2):
                    psh = pmm.tile([128, 1024], f32, tag="pmm")
                    for q in range(2):
                        rq = r0 + q * 8
                        nc.tensor.matmul(psh[:, bass.ts(q, 512)],
                                         ct["fc0a"][:, bass.ts(m, 128)],
                                         dxv[0:C2, rq + 1:rq + 9, 1:S + 1],
                                         start=True, stop=False)
                        nc.tensor.matmul(psh[:, bass.ts(q, 512)],
                                         ct["fc0bb"][:, bass.ts(m, 128)],
                                         yc[:, bass.ts(q, 512)],
                                         start=False, stop=True)
                    nc.scalar.activation(
                        h_tiles[m][:, bass.ts(T, 1024)], psh[:],
                        AF.Lrelu, bias=ct["fc0b2"][:, m:m + 1], scale=1.0,
                        alpha=0.01, accum_out=s1cols[:, T * 2 + m:T * 2 + m + 1])
            # sum of squares: half passes per m into 4 accum cols
            s2cols4 = spool.tile([128, 4], f32, tag="s2cols4")
            for m in range(2):
                for hh in range(2):
                    scr = sqpool.tile([128, NPIX // 2], bf16, tag="sqscr")
                    hs = h_tiles[m][:, bass.ts(hh, NPIX // 2)]
                    nc.vector.scalar_tensor_tensor(
                        out=scr[:], in0=hs, scalar=0.0, in1=hs,
                        op0=ALU.bypass, op1=ALU.mult,
                        accum_out=s2cols4[:, m * 2 + hh:m * 2 + hh + 1])
            stats2 = stpool.tile([128, 32], f32, tag="stats2", name=f"stats2_{b}")
            nc.vector.tensor_reduce(stats2[:, 0:1], s1cols[:], axis=mybir.AxisListType.X,
                                    op=ALU.add)
            nc.vector.tensor_reduce(stats2[:, 1:2], s2cols4[:], axis=mybir.AxisListType.X,
                                    op=ALU.add)
            # cross-partition reduce on DVE (no PE involvement): 32x32
            # block-transpose, in-quadrant reduce, quadrant gather, sum.
            tt = stpool.tile([128, 32], f32, tag="statt")
            nc.vector.transpose(tt[:], stats2[:])
            rq = spool.tile([128, 1], f32, tag="statq")
            nc.vector.tensor_reduce(rq[:], tt[:], axis=mybir.AxisListType.X,
                                    op=ALU.add)
            qp = spool.tile([32, 4], f32, tag="qacc")
            for j in range(4):
                nc.vector.stream_shuffle(qp[:, j:j + 1], rq[32 * j:32 * j + 32, :],
                                         list(range(32)))
            tot = spool.tile([32, 1], f32, tag="qtot")
            nc.vector.tensor_reduce(tot[:], qp[:], axis=mybir.AxisListType.X,
                                    op=ALU.add)
            gpd2.dma_start(DS[b][:], tot[0:2, 0])
            bc = spool.tile([128, 2], f32, tag="bcst", name=f"bc_{b}")
            gpd2.dma_start(bc[:], bass.AP(DS, b * 2, [[0, 128], [1, 2]]))
            if b % 2 == 0:
                muvp = spool.tile([128, 4], f32, tag="muvp", name=f"muvp_{b // 2}")
                stats[b // 2] = muvp
            else:
                muvp = stats[b // 2]
            hb = b % 2
            mu = muvp[:, hb:hb + 1]
            nc.vector.tensor_scalar_mul(mu, bc[:, 0:1], 1.0 / LN_N)
            msq = spool.tile([128, 1], f32, tag="stat")
            nc.vector.tensor_mul(msq[:], mu, mu)
            var = muvp[:, 2 + hb:3 + hb]
            nc.vector.scalar_tensor_tensor(out=var, in0=bc[:, 1:2],
                                           scalar=1.0 / LN_N, in1=msq[:],
                                           op0=ALU.mult, op1=ALU.subtract)
            nc.vector.tensor_scalar_add(var, var, EPS)
            for m in range(2):
                nc.vector.tensor_mul(h_tiles[m][:], h_tiles[m][:],
                               ct["lnw"][:, bass.ts(m, NPIX)])

        def s_fc1(pair):
            b0, b1 = 2 * pair, 2 * pair + 1
            muvp = stats[pair]
            sd2 = spool.tile([128, 2], f32, tag="stat2")
            nc.scalar.activation(sd2[:], muvp[:, 2:4], AF.Sqrt, bias=0.0, scale=1.0)
            rp = spool.tile([128, 2], f32, tag="stat2")
            nc.vector.reciprocal(rp[:], sd2[:])
            r2 = spool.tile([128, 1], f32, tag="statr")
            nc.vector.tensor_copy(r2[0:64, :], rp[0:64, 0:1])
            nc.vector.tensor_copy(r2[64:128, :], rp[64:128, 1:2])
            mu2 = spool.tile([128, 1], f32, tag="statr")
            nc.vector.tensor_copy(mu2[0:64, :], muvp[0:64, 0:1])
            nc.vector.tensor_copy(mu2[64:128, :], muvp[64:128, 1:2])
            nrm2 = spool.tile([128, 1], f32, tag="statr")
            nc.vector.scalar_tensor_tensor(out=nrm2[:], in0=mu2[:], scalar=-1.0,
                                           in1=r2[:], op0=ALU.mult, op1=ALU.mult)
            z = zpool.tile([128, NPIX], bf16, tag="ztile")
            zeng = nc.gpsimd if bool(int(os.environ.get('GP_Z','0'))) else nc.vector
            zeng.scalar_tensor_tensor(
                out=z[:], in0=ct["lw1t"][:], scalar=nrm2[:], in1=ct["lbt"][:],
                op0=ALU.mult, op1=ALU.add)
            mask2 = mpool.tile([128, NPIX], bf16, tag="mask2", name=f"mask2_{pair}")
            gpd2.dma_start(mask2[:], maskd[pair][:])
            dm = dmpool.tile([128, NPIX], bf16, tag="dm", name=f"dm_{pair}")
            h0, h1 = st[b0]["h"], st[b1]["h"]
            for T in range(4):
                psda = pmm.tile([128, 1024], f32, tag="pmm", name=f"psda_{pair}_{T}")
                psdb = pmm.tile([128, 1024], f32, tag="pmm", name=f"psdb_{pair}_{T}")
                for q in range(2):
                    for m in range(2):
                        for half, hh, pt in ((0, h0, psda), (1, h1, psdb)):
                            nc.tensor.matmul(
                                pt[bass.ts(half, 64), bass.ts(q, 512)],
                                ct["fc1"][:, bass.ts(m, 64)],
                                hh[m][:, bass.ds(T * 1024 + q * 512, 512)],
                                start=(m == 0), stop=(m == 1),
                                tile_position=(0, half * 64))
                for half, pt in ((0, psda), (1, psdb)):
                    hs = bass.ts(half, 64)
                    nc.vector.scalar_tensor_tensor(
                        out=dm[hs, bass.ts(T, 1024)], in0=pt[hs, :],
                        scalar=r2[hs, :], in1=z[hs, bass.ts(T, 1024)],
                        op0=ALU.mult, op1=ALU.add)
            nc.vector.tensor_mul(dm[:], dm[:], mask2[:])
            for hb in range(2):
                for ri in range(2):
                    nc.sync.dma_start(
                        D3[pair][hb].rearrange("(ri u) (c v) -> ri c u v",
                                               ri=2, u=S, c=C, v=S)[ri],
                        dm[bass.ds(hb * 64 + ri * 32, 32), :].rearrange(
                            "c (u v) -> c u v", u=S, v=S))
            for half, b in ((0, b0), (1, b1)):
                dg = dgpool.tile([2 * S, C * S], bf16, tag="dg", name=f"dg_{b}")
                d3g = D3[pair][half].rearrange("(ri u) (c v) -> ri u c v",
                                               ri=2, u=S, c=C, v=S)
                for ri in range(2):
                    nc.sync.dma_start(
                        dg[bass.ts(ri, S), :].rearrange("p (c v) -> p c v", c=C, v=S),
                        d3g[ri])
                dgath[b] = dg

        def s_ia(b):
            upd = dgath[b]
            sa = sapool.tile([2 * S, C * S], bf16, tag="sa", name=f"sa_{b}")
            for half in range(2):
                ps = pfft.tile([2 * S, 1024], f32, tag="pfft")
                for q in range(2):
                    nc.tensor.matmul(ps[:, bass.ts(q, 512)], ct["wa"][:],
                                     upd[:, bass.ds(half * 1024 + q * 512, 512)])
                eng = nc.vector.tensor_copy if half == 0 else nc.scalar.copy
                eng(sa[:, bass.ts(half, 1024)], ps[:])
            nc.scalar.dma_start(
                D4[b][0:NFLAT].rearrange("(p f) -> p f", p=2 * S, f=C * S), sa[:])

        def s_tm(b):
            for ri in range(2):
                dgb = gbpool.tile([128, S * C], bf16, tag="dgb", name=f"dgb_{b}_{ri}")
                in_ap = bass.AP(D4, b * (NFLAT + 64) + ri * (NFLAT // 2),
                                [[64, 2048], [1, 128]])
                nc.scalar.dma_start(dgb[:], in_ap, transpose=True)
                st[b][f"dgb{ri}"] = dgb

        def s_ib(b):
            g0, g1 = st[b]["dgb0"], st[b]["dgb1"]
            sb = sbpool.tile([2 * S, S * C], bf16, tag="sb", name=f"sb_{b}")
            for half in range(2):
                ps = pfft.tile([2 * S, 1024], f32, tag="pfft")
                for q in range(2):
                    sl = bass.ds(half * 1024 + q * 512, 512)
                    nc.tensor.matmul(ps[:, bass.ts(q, 512)], ct["walo"][:],
                                     g0[0:S, sl], start=True, stop=False)
                    nc.tensor.matmul(ps[:, bass.ts(q, 512)], ct["wahi"][:],
                                     g1[0:S, sl], start=False, stop=True)
                eng = nc.vector.tensor_copy if half == 0 else nc.scalar.copy
                eng(sb[:, bass.ts(half, 1024)], ps[:])
            gpd2.dma_start(OUT[b][:], sb[:])

        assert steps == 1, "device program built for steps==1"

        # wavefront emission: stage s of batch b emitted in round t = b + s.
        STAGES = [(0, s_load), (1, s_f1), (2, s_ta), (3, s_f2), (4, s_dx),
                  (5, s_conv), (6, s_fc1), (8, s_ia), (9, s_tm), (10, s_ib)]
        for t in range(BPC + 11):
            for s, fn in sorted(STAGES, key=lambda x: -x[0]):
                b = t - s
                if not (0 <= b < BPC):
                    continue
                if fn is s_fc1:
                    if b % 2 == 1:
                        fn(b // 2)
                else:
                    fn(b)

    return nc


_BUILT = {}


def kernel(**inputs):
    x = np.ascontiguousarray(np.asarray(inputs["x"], dtype=np.float32))
    steps = int(np.asarray(inputs["steps"]))
    if steps == 0:
        return x.astype(np.complex64)
    assert steps == 1, f"unsupported steps={steps}"

    cst = host_constants(inputs)
    su = np.asarray(inputs["stoch_u"], dtype=np.float32)[..., 0]
    mask = (su > FIRE).astype(np.float32)
    mask_dev = np.ascontiguousarray(np.transpose(mask, (0, 2, 1))
                                    ).reshape(B, NPIX).astype(_BF)
    mask_pairs = np.empty((B // 2, 128, NPIX), _BF)
    for p in range(B // 2):
        mask_pairs[p, :64] = mask_dev[2 * p][None, :]
        mask_pairs[p, 64:] = mask_dev[2 * p + 1][None, :]

    # device x layout: [b, s1, c, s2]
    xdev = np.ascontiguousarray(np.transpose(x, (0, 1, 3, 2))).astype(_BF)

    if "nc" not in _BUILT:
        nc = build_nc(steps=1)
        nc.finalize()
        _BUILT["nc"] = nc
    nc = _BUILT["nc"]

    in_maps = []
    for core in range(NCORES):
        m = {k: np.ascontiguousarray(v) for k, v in cst.items()}
        m["xs"] = xdev[core * BPC:(core + 1) * BPC].reshape(BPC, S, C * S)
        m["maskd"] = mask_pairs[core * (BPC // 2):(core + 1) * (BPC // 2)]
        in_maps.append(m)

    from concourse.bass_utils import run_bass_kernel_spmd
    trace = bool(int(os.environ.get("KERNEL_TRACE", "0")))
    res = run_bass_kernel_spmd(nc, in_maps, list(range(NCORES)), trace=trace)
    if trace and res.exec_time_ns is not None:
        print(f"HW exec time: {res.exec_time_ns} ns")
        if res.instructions_and_trace is not None:
            print("trace:", res.instructions_and_trace[1])

    out = np.empty((B, S, S, C), np.complex64)
    for core in range(NCORES):
        o = np.asarray(res.results[core]["OUT"], dtype=np.float32)
        for j in range(BPC):
            b = core * BPC + j
            re = o[j, :S].reshape(S, S, C)
            im = o[j, S:].reshape(S, S, C)
            out[b] = x[b] + re + 1j * im
    return out
